# revision 14
# baseline (speedup 1.0000x reference)
"""Trainium2 Bass kernel for BaseAttention (B=4, S=2048, H=16 heads x 64).

Sharding: 8 cores = 4 batches x 2 head-groups (8 heads / 512 dims each).
Each core computes q/k/v projections for its head group on its batch,
flash-style causal attention (scores never leave the chip), and a partial
o-projection over its 512 head dims. The host sums the two partial outputs
per batch.

All matmul operands are bf16 (inputs cast on host): the PE streams one
moving column per cycle regardless of dtype, but bf16 halves SBUF/DMA
traffic and weight-load time. Accumulation stays fp32 in PSUM; measured
end-to-end error vs the fp32 reference is ~5e-3 max-rel. The attention
matrix never leaves SBUF: normalized probs land in attnT_sb (bf16) and
the o-projection consumes them directly.

Softmax denominators come from a ones column appended to V; normalization
broadcasts the denominator row with a K=1 matmul, takes the reciprocal on
DVE, and scales attn^T while writing bf16 into attnT_sb.
"""

import numpy as np

B = 4
S = 2048
HIDDEN = 1024
NH = 16
DH = 64
HG = 2                  # head groups (cores per batch)
DG = HIDDEN // HG       # 512 dims per group (8 heads)
NCORES = B * HG
SCALE = DH ** -0.5

P = 128
KC = HIDDEN // P        # 8 contraction chunks for projections
NQ = S // 512           # 4 query chunks of 512
SM = S // P             # 16 seq chunks of 128
MCH = DG // P           # 4 chunks of 128 over the group's 512 dims
NHG = NH // HG          # 8 heads per core
NJ = NHG // 2           # 4 head pairs per core

_CACHE = {}


def _emit(nc, tc, tens):
    import concourse.mybir as mybir
    import concourse.bass as bass
    from collections import deque
    from contextlib import ExitStack

    f32 = mybir.dt.float32
    f32r = mybir.dt.float32r
    bf16 = mybir.dt.bfloat16
    Exp = mybir.ActivationFunctionType.Exp
    mult = mybir.AluOpType.mult
    ds = bass.ds

    xT = tens["xT"].ap().rearrange("(kc p) s -> p kc s", p=P)
    wqT = tens["wqT"].ap().rearrange("(kc p) d -> p kc d", p=P)
    wkT = tens["wkT"].ap().rearrange("(kc p) d -> p kc d", p=P)
    wvT = tens["wvT"].ap().rearrange("(kc p) d -> p kc d", p=P)
    woT = tens["woT"].ap().rearrange("(ic p) j -> p ic j", p=P)
    masks = tens["masks"].ap().rearrange("t p q -> p t q")
    out = tens["out"].ap().rearrange("(sm p) j -> p sm j", p=P)

    with ExitStack() as ctx:
        persist = ctx.enter_context(tc.tile_pool(name="persist", bufs=1))
        ps_mm = ctx.enter_context(tc.tile_pool(name="ps_mm", bufs=2, space="PSUM"))
        ps_at = ctx.enter_context(tc.tile_pool(name="ps_at", bufs=4, space="PSUM"))
        pstage = ctx.enter_context(tc.tile_pool(name="pstage", bufs=2))
        ptp = ctx.enter_context(tc.tile_pool(name="pt", bufs=4))
        recp = ctx.enter_context(tc.tile_pool(name="rec", bufs=2))
        ostp = ctx.enter_context(tc.tile_pool(name="ost", bufs=3))
        qpool = ctx.enter_context(tc.tile_pool(name="qp", bufs=2))

        kT_sb = persist.tile([P, MCH, S], bf16)          # k^T (d on partitions)
        v_sb = persist.tile([P, SM, NHG, DH + 1], bf16)  # v + ones column
        ones_sb = persist.tile([P, DH], f32r)
        wq_sb = persist.tile([P, KC, DG], bf16)
        wk_sb = persist.tile([P, KC, DG], bf16)
        wv_sb = persist.tile([P, KC, DG], bf16)
        wo_sb = persist.tile([P, MCH, HIDDEN], bf16)
        mask_sb = persist.tile([P, 2, 1024], bf16)
        attnT_sb = persist.tile([P, MCH, S], bf16)       # normalized attn^T

        ones_f32 = persist.tile([P, 1], f32)
        nc.vector.memset(ones_f32[:], 1.0)  # f32r memset fails ISA checks
        nc.vector.tensor_copy(out=ones_sb[:], in_=ones_f32[:, 0:1].to_broadcast([P, DH]))
        nc.vector.tensor_copy(
            out=v_sb[:, :, :, DH:DH + 1],
            in_=ones_f32[:, 0:1].to_broadcast([P, SM, NHG, 1]),
        )

        xts = {}
        qsbs = {}

        def proj_closures(n):
            """q/k/v projection work for seq chunk n: 13 closures."""
            cls = []

            def load_xt(n=n):
                xt = pstage.tile([P, KC, 512], bf16, tag="xt")
                # fine-grained: one DMA per kc (128KB) for queue balance and
                # so the first q chain can start as soon as early chunks land
                for kc in range(KC):
                    nc.sync.dma_start(xt[:, kc, :], xT[:, kc, ds(n * 512, 512)])
                xts[n] = xt
                qsbs[n] = qpool.tile([P, MCH, 512], bf16, tag="qsb", name=f"qsb_{n}")
            cls.append(load_xt)

            for w_sb, dst_name in ((wq_sb, "q"), (wk_sb, "k")):
                for m in range(MCH):
                    def qk_group(n=n, w_sb=w_sb, dst_name=dst_name, m=m):
                        xt = xts[n]
                        ps = ps_mm.tile([P, 1024], f32, tag="mm")
                        for kc in range(KC):
                            nc.tensor.matmul(
                                ps[:, :512],
                                w_sb[:, kc, ds(m * P, P)],
                                xt[:, kc, :],
                                start=(kc == 0), stop=(kc == KC - 1),
                            )
                        if dst_name == "k":
                            nc.vector.tensor_copy(
                                out=kT_sb[:, m, ds(n * 512, 512)], in_=ps[:, :512]
                            )
                        else:
                            nc.vector.tensor_copy(
                                out=qsbs[n][:, m, :], in_=ps[:, :512]
                            )
                    cls.append(qk_group)

            for sm in range(4 * n, 4 * n + 4):
                def v_group(n=n, sm=sm):
                    xt = xts[n]
                    ps = ps_mm.tile([P, 1024], f32, tag="mm")
                    for kc in range(KC):
                        nc.tensor.matmul(
                            ps[:, :512],
                            xt[:, kc, ds((sm - 4 * n) * P, P)],
                            wv_sb[:, kc, :],
                            start=(kc == 0), stop=(kc == KC - 1),
                        )
                    nc.vector.tensor_copy(
                        out=v_sb[:, sm, :, 0:DH],
                        in_=ps[:, :512].rearrange("p (h d) -> p h d", h=NHG),
                    )
                cls.append(v_group)
            return cls

        def oproj_closures(n):
            """o-projection for seq chunk n, reading attn^T from SBUF."""
            cls = []
            for sm in range(4 * n, 4 * n + 4):
                for j2 in range(2):
                    def o_group(sm=sm, j2=j2):
                        ps = ps_mm.tile([P, 1024], f32, tag="mm")
                        for ic in range(MCH):
                            nc.tensor.matmul(
                                ps[:, :512],
                                attnT_sb[:, ic, ds(sm * P, P)],
                                wo_sb[:, ic, ds(j2 * 512, 512)],
                                start=(ic == 0), stop=(ic == MCH - 1),
                            )
                        ost = ostp.tile([P, 512], f32, tag="ost")
                        nc.vector.tensor_copy(out=ost[:], in_=ps[:, :512])
                        nc.sync.dma_start(out[:, sm, ds(j2 * 512, 512)], ost[:])
                    cls.append(o_group)
            return cls

        # startup: xt(0)+wq first so the PE starts within a few us; the
        # remaining weight DMAs stream behind the first matmul groups.
        def wload(dst, src, ncol, nsplit, kdim=KC):
            # split a [*, kdim, ncol] weight load into 128KB pieces,
            # column-major so earlier-needed columns land first
            for c in range(ncol // P):
                csl = ds(c * P, P)
                for h in range(nsplit):
                    ksl = ds(h * (kdim // nsplit), kdim // nsplit)
                    nc.sync.dma_start(dst[:, ksl, csl], src[:, ksl, csl])

        # startup: issue DMAs in consumption order, 128KB pieces so no
        # single queue serializes a dependency (q m1 @~4us, mask @~6us, ...)
        p0 = proj_closures(0)
        p0[0]()                                   # xt(0) + qsb alloc
        wload(wq_sb, wqT, P, 2)                   # wq m0 column only
        p0[1]()                                   # q m0 group
        for c4 in range(1, 4):
            csl = ds(c4 * P, P)
            for h2 in range(2):
                ksl = ds(h2 * (KC // 2), KC // 2)
                nc.sync.dma_start(wq_sb[:, ksl, csl], wqT[:, ksl, csl])
        for q4 in range(4):
            nc.sync.dma_start(mask_sb[:, :, ds(q4 * 256, 256)],
                              masks[:, :, ds(q4 * 256, 256)])
        for c in p0[2:5]:                         # q m1..3 groups
            c()
        wload(wk_sb, wkT, DG, 2)
        for c in p0[5:9]:                         # k groups
            c()
        wload(wv_sb, wvT, DG, 2)
        for c in p0[9:]:                          # v groups
            c()
        wload(wo_sb, woT, HIDDEN, 1, kdim=MCH)

        filler = deque()
        pending = []  # deferred normalization closures

        def flush_pending():
            for c in pending:
                c()
            pending.clear()

        def norm_closure(n, j, acc):
            qsl = ds(n * 512, 512)
            # copy both raw denominator rows to SBUF (f32r) as bcast-mm rhs
            rec = recp.tile([DH + 1, 1024], f32r, tag="rec")
            with nc.allow_low_precision(reason="denom row stage"):
                for e in range(2):
                    nc.vector.tensor_copy(
                        out=rec[DH:DH + 1, ds(e * 512, 512)],
                        in_=acc[e][DH:DH + 1, :],
                    )

            def finish():
                bc = ps_mm.tile([P, 1024], f32, tag="mm", name=f"bc_{n}_{j}")
                for e in range(2):
                    nc.tensor.matmul(
                        bc[0:DH, ds(e * 512, 512)], ones_sb[DH:DH + 1, :],
                        rec[DH:DH + 1, ds(e * 512, 512)], start=True, stop=True,
                    )
                rcp = recp.tile([DH, 1024], f32, tag="rcp")
                nc.vector.reciprocal_approx_fast(rcp[:], bc[0:DH, :])
                with nc.allow_low_precision(reason="attn bf16 store"):
                    for e in range(2):
                        nc.vector.tensor_tensor(
                            attnT_sb[ds(64 * e, DH), j, qsl], acc[e][0:DH, :],
                            rcp[:, ds(e * 512, 512)], mult,
                        )
            return finish

        for n in range(NQ):
            if n + 1 < NQ:
                pc = proj_closures(n + 1)
                pc[0]()                       # start xt(n+1) DMA immediately
                filler.extend(pc[1:])
            if 1 <= n <= 3:
                filler.extend(oproj_closures(n - 1))
            npairs = 2 * (n + 1)
            total_pairs = NJ * npairs
            pace_num = len(filler)
            pace_acc = 0
            for j in range(NJ):
                acc = [
                    ps_at.tile([DH + 1, 512], f32, tag="acc",
                               name=f"acc_{n}_{j}_{e}")
                    for e in range(2)
                ]
                pvq = []  # deferred PV matmuls (consumed 2 pairs later)

                def emit_pv():
                    tp, e, u, pt, qoff = pvq.pop(0)
                    h = 2 * j + e
                    m = 2 * tp + u
                    nc.tensor.matmul(
                        acc[e][:, qoff:512],
                        v_sb[:, m, h, :],
                        pt[:, ds(u * 512 + qoff, 512 - qoff)],
                        start=(tp == 0 and u == 0),
                        stop=(tp == npairs - 1 and u == 1),
                        skip_group_check=(qoff > 0),
                    )

                for t in range(npairs):
                    if pending:
                        flush_pending()
                    # pump interleaved proj/o-proj work in bursts of >=2
                    # groups: a dense >3.4us PE stretch lets the HAM clock
                    # gate open (scattered 1-group pumps never do)
                    pace_acc += pace_num
                    while pace_acc >= total_pairs and filler:
                        filler.popleft()()
                        pace_acc -= total_pairs
                    diag = t - 2 * n            # >=0: diagonal pair
                    # causal trim: within the diagonal 512-block, q columns
                    # strictly left of a key block are fully masked — skip
                    # their scores/exp/PV entirely (start flags still cover
                    # the full width via the t==0,u==0 PV).
                    qoffs = ((0, 128) if diag == 0 else
                             (256, 384) if diag == 1 else (0, 0))
                    new_pvq = []
                    for e in range(2):          # head pair member
                        bp = e * DH             # base partition 0/64
                        ps = ps_mm.tile([P, 1024], f32, tag="mm")
                        for u in range(2):      # m-pair member
                            m = 2 * t + u
                            qo = qoffs[u]
                            nc.tensor.matmul(
                                ps[:, ds(u * 512 + qo, 512 - qo)],
                                kT_sb[bp:bp + DH, j, ds(m * P, P)],
                                qsbs[n][bp:bp + DH, j, ds(qo, 512 - qo)],
                                start=True, stop=True,
                            )
                            if pvq and pvq[0][0] <= t - 2:
                                emit_pv()
                        pt = ptp.tile([P, 1024], bf16, tag="pt")
                        if diag == 1:
                            nc.scalar.activation(pt[:, 256:512], ps[:, 256:512],
                                                 Exp, scale=SCALE)
                            nc.scalar.activation(pt[:, 896:1024], ps[:, 896:1024],
                                                 Exp, scale=SCALE)
                        else:
                            nc.scalar.activation(pt[:], ps[:], Exp, scale=SCALE)
                        if diag == 0:
                            nc.vector.tensor_tensor(
                                pt[:, 0:128], pt[:, 0:128],
                                mask_sb[:, 0, 0:128], mult)
                            nc.vector.tensor_tensor(
                                pt[:, 640:768], pt[:, 640:768],
                                mask_sb[:, 0, 640:768], mult)
                        elif diag == 1:
                            nc.vector.tensor_tensor(
                                pt[:, 256:384], pt[:, 256:384],
                                mask_sb[:, 1, 256:384], mult)
                            nc.vector.tensor_tensor(
                                pt[:, 896:1024], pt[:, 896:1024],
                                mask_sb[:, 1, 896:1024], mult)
                        if pvq and pvq[0][0] <= t - 2:
                            emit_pv()
                        new_pvq.extend((t, e, u, pt, qoffs[u]) for u in range(2))
                    while pvq and pvq[0][0] <= t - 2:
                        emit_pv()
                    pvq.extend(new_pvq)
                while pvq:
                    emit_pv()
                pending.append(norm_closure(n, j, acc))
            while filler:
                filler.popleft()()
        flush_pending()
        for c in oproj_closures(NQ - 1):
            c()


def _build():
    import concourse.mybir as mybir
    import concourse.tile as tile
    from concourse import bacc

    f32 = mybir.dt.float32
    bf16 = mybir.dt.bfloat16
    nc = bacc.Bacc("TRN2", target_bir_lowering=False, debug=False,
                   num_devices=NCORES)
    tens = {
        "xT": nc.dram_tensor("xT", [HIDDEN, S], bf16, kind="ExternalInput"),
        "wqT": nc.dram_tensor("wqT", [HIDDEN, DG], bf16, kind="ExternalInput"),
        "wkT": nc.dram_tensor("wkT", [HIDDEN, DG], bf16, kind="ExternalInput"),
        "wvT": nc.dram_tensor("wvT", [HIDDEN, DG], bf16, kind="ExternalInput"),
        "woT": nc.dram_tensor("woT", [DG, HIDDEN], bf16, kind="ExternalInput"),
        "masks": nc.dram_tensor("masks", [2, P, 1024], bf16, kind="ExternalInput"),
        "out": nc.dram_tensor("out", [S, HIDDEN], f32, kind="ExternalOutput"),
    }
    with tile.TileContext(nc) as tc:
        _emit(nc, tc, tens)
    nc.compile()
    return nc


def get_program():
    if "nc" not in _CACHE:
        _CACHE["nc"] = _build()
    return _CACHE["nc"]


def make_in_maps(hidden_states, attention_mask, wq, wk, wv, wo):
    """Build the per-core input maps (host-side sharding, bf16 cast)."""
    import ml_dtypes
    bf = ml_dtypes.bfloat16
    hidden_states = np.asarray(hidden_states, dtype=np.float32)
    attention_mask = np.asarray(attention_mask, dtype=np.float32)
    wq = np.asarray(wq, dtype=np.float32)
    wk = np.asarray(wk, dtype=np.float32)
    wv = np.asarray(wv, dtype=np.float32)
    wo = np.asarray(wo, dtype=np.float32)

    # Pair-level mask tiles for the diagonal blocks of scores^T, derived from
    # the provided additive mask (0 = attend, big negative = blocked).
    # Tile [t][kk, 512u + qq] = allow(q = 512 + qq, k = 512 + (2t+u)*128 + kk).
    am = attention_mask[0, 0]
    mask_np = np.empty((2, P, 1024), dtype=np.float32)
    for t in range(2):
        for u in range(2):
            off = (2 * t + u) * P
            blk = (am[512:1024, 512 + off:512 + off + P] == 0.0)
            mask_np[t, :, u * 512:(u + 1) * 512] = blk.T.astype(np.float32)
    mask_np = mask_np.astype(bf)

    in_maps = []
    for c in range(NCORES):
        b, g = divmod(c, HG)
        rows = slice(g * DG, (g + 1) * DG)
        in_maps.append({
            "xT": np.ascontiguousarray(hidden_states[b].T).astype(bf),
            "wqT": np.ascontiguousarray(wq[rows, :].T).astype(bf),
            "wkT": np.ascontiguousarray(wk[rows, :].T).astype(bf),
            "wvT": np.ascontiguousarray(wv[rows, :].T).astype(bf),
            "woT": np.ascontiguousarray(wo[:, rows].T).astype(bf),
            "masks": mask_np,
        })
    return in_maps


def combine_outputs(results):
    out = np.empty((B, S, HIDDEN), dtype=np.float32)
    for b in range(B):
        out[b] = results[HG * b]["out"] + results[HG * b + 1]["out"]
    return out


def kernel(hidden_states, attention_mask, wq, wk, wv, wo):
    from concourse.bass_utils import run_bass_kernel_spmd

    nc = get_program()
    in_maps = make_in_maps(hidden_states, attention_mask, wq, wk, wv, wo)
    res = run_bass_kernel_spmd(nc, in_maps, list(range(NCORES)))
    return combine_outputs(res.results)


# revision 15
# speedup vs baseline: 1.1251x; 1.1251x over previous
"""Trainium2 Bass kernel for BaseAttention (B=4, S=2048, H=16 heads x 64).

Sharding: 8 cores = 4 batches x 2 head-groups (8 heads / 512 dims each).
Each core computes q/k/v projections for its head group on its batch,
flash-style causal attention (scores never leave the chip), and a partial
o-projection over its 512 head dims. The host sums the two partial outputs
per batch.

All matmul operands are bf16 (inputs cast on host): the PE streams one
moving column per cycle regardless of dtype, but bf16 halves SBUF/DMA
traffic and weight-load time. Accumulation stays fp32 in PSUM; measured
end-to-end error vs the fp32 reference is ~5e-3 max-rel. The attention
matrix never leaves SBUF: normalized probs land in attnT_sb (bf16) and
the o-projection consumes them directly.

Softmax denominators come from a ones column appended to V; normalization
broadcasts the denominator row with a K=1 matmul, takes the reciprocal on
DVE, and scales attn^T while writing bf16 into attnT_sb.
"""

import numpy as np

B = 4
S = 2048
HIDDEN = 1024
NH = 16
DH = 64
HG = 2                  # head groups (cores per batch)
DG = HIDDEN // HG       # 512 dims per group (8 heads)
NCORES = B * HG
SCALE = DH ** -0.5

P = 128
KC = HIDDEN // P        # 8 contraction chunks for projections
NQ = S // 512           # 4 query chunks of 512
SM = S // P             # 16 seq chunks of 128
MCH = DG // P           # 4 chunks of 128 over the group's 512 dims
NHG = NH // HG          # 8 heads per core
NJ = NHG // 2           # 4 head pairs per core

_CACHE = {}


def _emit(nc, tc, tens):
    import concourse.mybir as mybir
    import concourse.bass as bass
    from collections import deque
    from contextlib import ExitStack

    f32 = mybir.dt.float32
    f32r = mybir.dt.float32r
    bf16 = mybir.dt.bfloat16
    Exp = mybir.ActivationFunctionType.Exp
    mult = mybir.AluOpType.mult
    ds = bass.ds

    xT = tens["xT"].ap().rearrange("(kc p) s -> p kc s", p=P)
    wqT = tens["wqT"].ap().rearrange("(kc p) d -> p kc d", p=P)
    wkT = tens["wkT"].ap().rearrange("(kc p) d -> p kc d", p=P)
    wvT = tens["wvT"].ap().rearrange("(kc p) d -> p kc d", p=P)
    woT = tens["woT"].ap().rearrange("(ic p) j -> p ic j", p=P)
    masks = tens["masks"].ap().rearrange("t p q -> p t q")
    out = tens["out"].ap().rearrange("(sm p) j -> p sm j", p=P)

    with ExitStack() as ctx:
        persist = ctx.enter_context(tc.tile_pool(name="persist", bufs=1))
        ps_mm = ctx.enter_context(tc.tile_pool(name="ps_mm", bufs=3, space="PSUM"))
        ps_at = ctx.enter_context(tc.tile_pool(name="ps_at", bufs=2, space="PSUM"))
        pstage = ctx.enter_context(tc.tile_pool(name="pstage", bufs=2))
        ptp = ctx.enter_context(tc.tile_pool(name="pt", bufs=4))
        recp = ctx.enter_context(tc.tile_pool(name="rec", bufs=2))
        ostp = ctx.enter_context(tc.tile_pool(name="ost", bufs=3))
        qpool = ctx.enter_context(tc.tile_pool(name="qp", bufs=2))

        kT_sb = persist.tile([P, MCH, S], bf16)          # k^T (d on partitions)
        v_sb = persist.tile([P, SM, NHG, DH + 1], bf16)  # v + ones column
        ones_sb = persist.tile([P, DH], f32r)
        wq_sb = persist.tile([P, KC, DG], bf16)
        wk_sb = persist.tile([P, KC, DG], bf16)
        wv_sb = persist.tile([P, KC, DG], bf16)
        wo_sb = persist.tile([P, MCH, HIDDEN], bf16)
        mask_sb = persist.tile([P, 2, 1024], bf16)
        attnT_sb = persist.tile([P, MCH, S], bf16)       # normalized attn^T

        ones_f32 = persist.tile([P, 1], f32)
        nc.vector.memset(ones_f32[:], 1.0)  # f32r memset fails ISA checks
        nc.vector.tensor_copy(out=ones_sb[:], in_=ones_f32[:, 0:1].to_broadcast([P, DH]))
        nc.vector.tensor_copy(
            out=v_sb[:, :, :, DH:DH + 1],
            in_=ones_f32[:, 0:1].to_broadcast([P, SM, NHG, 1]),
        )

        xts = {}
        qsbs = {}

        def proj_closures(n):
            """q/k/v projection work for seq chunk n: 13 closures."""
            cls = []

            def load_xt(n=n):
                xt = pstage.tile([P, KC, 512], bf16, tag="xt")
                # fine-grained: one DMA per kc (128KB) for queue balance and
                # so the first q chain can start as soon as early chunks land
                for kc in range(KC):
                    nc.sync.dma_start(xt[:, kc, :], xT[:, kc, ds(n * 512, 512)])
                xts[n] = xt
                qsbs[n] = qpool.tile([P, MCH, 512], bf16, tag="qsb", name=f"qsb_{n}")
            cls.append(load_xt)

            for w_sb, dst_name in ((wq_sb, "q"), (wk_sb, "k")):
                for m in range(MCH):
                    def qk_group(n=n, w_sb=w_sb, dst_name=dst_name, m=m):
                        xt = xts[n]
                        ps = ps_mm.tile([P, 1024], f32, tag="mm")
                        for kc in range(KC):
                            nc.tensor.matmul(
                                ps[:, :512],
                                w_sb[:, kc, ds(m * P, P)],
                                xt[:, kc, :],
                                start=(kc == 0), stop=(kc == KC - 1),
                            )
                        if dst_name == "k":
                            nc.vector.tensor_copy(
                                out=kT_sb[:, m, ds(n * 512, 512)], in_=ps[:, :512]
                            )
                        else:
                            nc.vector.tensor_copy(
                                out=qsbs[n][:, m, :], in_=ps[:, :512]
                            )
                    cls.append(qk_group)

            for sm in range(4 * n, 4 * n + 4):
                def v_group(n=n, sm=sm):
                    xt = xts[n]
                    ps = ps_mm.tile([P, 1024], f32, tag="mm")
                    for kc in range(KC):
                        nc.tensor.matmul(
                            ps[:, :512],
                            xt[:, kc, ds((sm - 4 * n) * P, P)],
                            wv_sb[:, kc, :],
                            start=(kc == 0), stop=(kc == KC - 1),
                        )
                    nc.vector.tensor_copy(
                        out=v_sb[:, sm, :, 0:DH],
                        in_=ps[:, :512].rearrange("p (h d) -> p h d", h=NHG),
                    )
                cls.append(v_group)
            return cls

        def oproj_closures(n):
            """o-projection for seq chunk n, reading attn^T from SBUF."""
            cls = []
            for sm in range(4 * n, 4 * n + 4):
                for j2 in range(2):
                    def o_group(sm=sm, j2=j2):
                        ps = ps_mm.tile([P, 1024], f32, tag="mm")
                        for ic in range(MCH):
                            nc.tensor.matmul(
                                ps[:, :512],
                                attnT_sb[:, ic, ds(sm * P, P)],
                                wo_sb[:, ic, ds(j2 * 512, 512)],
                                start=(ic == 0), stop=(ic == MCH - 1),
                            )
                        ost = ostp.tile([P, 512], f32, tag="ost")
                        nc.vector.tensor_copy(out=ost[:], in_=ps[:, :512])
                        nc.sync.dma_start(out[:, sm, ds(j2 * 512, 512)], ost[:])
                    cls.append(o_group)
            return cls

        # startup: xt(0)+wq first so the PE starts within a few us; the
        # remaining weight DMAs stream behind the first matmul groups.
        def wload(dst, src, ncol, nsplit, kdim=KC):
            # split a [*, kdim, ncol] weight load into 128KB pieces,
            # column-major so earlier-needed columns land first
            for c in range(ncol // P):
                csl = ds(c * P, P)
                for h in range(nsplit):
                    ksl = ds(h * (kdim // nsplit), kdim // nsplit)
                    nc.sync.dma_start(dst[:, ksl, csl], src[:, ksl, csl])

        # startup: issue DMAs in consumption order, 128KB pieces so no
        # single queue serializes a dependency (q m1 @~4us, mask @~6us, ...)
        p0 = proj_closures(0)
        p0[0]()                                   # xt(0) + qsb alloc
        wload(wq_sb, wqT, P, 2)                   # wq m0 column only
        p0[1]()                                   # q m0 group
        for c4 in range(1, 4):
            csl = ds(c4 * P, P)
            for h2 in range(2):
                ksl = ds(h2 * (KC // 2), KC // 2)
                nc.sync.dma_start(wq_sb[:, ksl, csl], wqT[:, ksl, csl])
        for q4 in range(4):
            nc.sync.dma_start(mask_sb[:, :, ds(q4 * 256, 256)],
                              masks[:, :, ds(q4 * 256, 256)])
        for c in p0[2:5]:                         # q m1..3 groups
            c()
        wload(wk_sb, wkT, DG, 2)
        for c in p0[5:9]:                         # k groups
            c()
        wload(wv_sb, wvT, DG, 2)
        for c in p0[9:]:                          # v groups
            c()
        wload(wo_sb, woT, HIDDEN, 1, kdim=MCH)

        filler = deque()
        pending = []  # deferred normalization closures

        def flush_pending():
            for c in pending:
                c()
            pending.clear()

        def norm_closure(n, j, acc):
            qsl = ds(n * 512, 512)
            # copy both raw denominator rows to SBUF (f32r) as bcast-mm rhs
            rec = recp.tile([DH + 1, 1024], f32r, tag="rec")
            with nc.allow_low_precision(reason="denom row stage"):
                for e in range(2):
                    nc.vector.tensor_copy(
                        out=rec[DH:DH + 1, ds(e * 512, 512)],
                        in_=acc[e][DH:DH + 1, :],
                    )

            def finish():
                bc = ps_mm.tile([P, 1024], f32, tag="mm", name=f"bc_{n}_{j}")
                for e in range(2):
                    nc.tensor.matmul(
                        bc[0:DH, ds(e * 512, 512)], ones_sb[DH:DH + 1, :],
                        rec[DH:DH + 1, ds(e * 512, 512)], start=True, stop=True,
                    )
                rcp = recp.tile([DH, 1024], f32, tag="rcp")
                nc.vector.reciprocal_approx_fast(rcp[:], bc[0:DH, :])
                with nc.allow_low_precision(reason="attn bf16 store"):
                    for e in range(2):
                        nc.vector.tensor_tensor(
                            attnT_sb[ds(64 * e, DH), j, qsl], acc[e][0:DH, :],
                            rcp[:, ds(e * 512, 512)], mult,
                        )
            return finish

        for n in range(NQ):
            if n + 1 < NQ:
                pc = proj_closures(n + 1)
                pc[0]()                       # start xt(n+1) DMA immediately
                filler.extend(pc[1:])
            if 1 <= n <= 3:
                filler.extend(oproj_closures(n - 1))
            npairs = 2 * (n + 1)
            total_pairs = NJ * npairs
            pace_num = len(filler)
            pace_acc = 0
            for j in range(NJ):
                acc = [
                    ps_at.tile([DH + 1, 512], f32, tag="acc",
                               name=f"acc_{n}_{j}_{e}")
                    for e in range(2)
                ]
                pvq = []  # deferred PV matmuls (consumed 2 pairs later)

                def emit_pv():
                    tp, e, u, pt, qoff = pvq.pop(0)
                    h = 2 * j + e
                    m = 2 * tp + u
                    nc.tensor.matmul(
                        acc[e][:, qoff:512],
                        v_sb[:, m, h, :],
                        pt[:, ds(u * 512 + qoff, 512 - qoff)],
                        start=(tp == 0 and u == 0),
                        stop=(tp == npairs - 1 and u == 1),
                        skip_group_check=(qoff > 0),
                    )

                for t in range(npairs):
                    if pending:
                        flush_pending()
                    # pump interleaved proj/o-proj work in bursts of >=2
                    # groups: a dense >3.4us PE stretch lets the HAM clock
                    # gate open (scattered 1-group pumps never do)
                    pace_acc += pace_num
                    while pace_acc >= total_pairs and filler:
                        filler.popleft()()
                        pace_acc -= total_pairs
                    diag = t - 2 * n            # >=0: diagonal pair
                    # causal trim: within the diagonal 512-block, q columns
                    # strictly left of a key block are fully masked — skip
                    # their scores/exp/PV entirely (start flags still cover
                    # the full width via the t==0,u==0 PV).
                    qoffs = ((0, 128) if diag == 0 else
                             (256, 384) if diag == 1 else (0, 0))
                    new_pvq = []
                    for e in range(2):          # head pair member
                        bp = e * DH             # base partition 0/64
                        ps = ps_mm.tile([P, 1024], f32, tag="mm")
                        for u in range(2):      # m-pair member
                            m = 2 * t + u
                            qo = qoffs[u]
                            nc.tensor.matmul(
                                ps[:, ds(u * 512 + qo, 512 - qo)],
                                kT_sb[bp:bp + DH, j, ds(m * P, P)],
                                qsbs[n][bp:bp + DH, j, ds(qo, 512 - qo)],
                                start=True, stop=True,
                            )
                            if pvq and pvq[0][0] <= t - 2:
                                emit_pv()
                        pt = ptp.tile([P, 1024], bf16, tag="pt")
                        if diag == 1:
                            nc.scalar.activation(pt[:, 256:512], ps[:, 256:512],
                                                 Exp, scale=SCALE)
                            nc.scalar.activation(pt[:, 896:1024], ps[:, 896:1024],
                                                 Exp, scale=SCALE)
                        else:
                            nc.scalar.activation(pt[:], ps[:], Exp, scale=SCALE)
                        if diag == 0:
                            nc.vector.tensor_tensor(
                                pt[:, 0:128], pt[:, 0:128],
                                mask_sb[:, 0, 0:128], mult)
                            nc.vector.tensor_tensor(
                                pt[:, 640:768], pt[:, 640:768],
                                mask_sb[:, 0, 640:768], mult)
                        elif diag == 1:
                            nc.vector.tensor_tensor(
                                pt[:, 256:384], pt[:, 256:384],
                                mask_sb[:, 1, 256:384], mult)
                            nc.vector.tensor_tensor(
                                pt[:, 896:1024], pt[:, 896:1024],
                                mask_sb[:, 1, 896:1024], mult)
                        if pvq and pvq[0][0] <= t - 2:
                            emit_pv()
                        new_pvq.extend((t, e, u, pt, qoffs[u]) for u in range(2))
                    while pvq and pvq[0][0] <= t - 2:
                        emit_pv()
                    pvq.extend(new_pvq)
                while pvq:
                    emit_pv()
                pending.append(norm_closure(n, j, acc))
            while filler:
                filler.popleft()()
        flush_pending()
        for c in oproj_closures(NQ - 1):
            c()


def _build():
    import concourse.mybir as mybir
    import concourse.tile as tile
    from concourse import bacc

    f32 = mybir.dt.float32
    bf16 = mybir.dt.bfloat16
    nc = bacc.Bacc("TRN2", target_bir_lowering=False, debug=False,
                   num_devices=NCORES)
    tens = {
        "xT": nc.dram_tensor("xT", [HIDDEN, S], bf16, kind="ExternalInput"),
        "wqT": nc.dram_tensor("wqT", [HIDDEN, DG], bf16, kind="ExternalInput"),
        "wkT": nc.dram_tensor("wkT", [HIDDEN, DG], bf16, kind="ExternalInput"),
        "wvT": nc.dram_tensor("wvT", [HIDDEN, DG], bf16, kind="ExternalInput"),
        "woT": nc.dram_tensor("woT", [DG, HIDDEN], bf16, kind="ExternalInput"),
        "masks": nc.dram_tensor("masks", [2, P, 1024], bf16, kind="ExternalInput"),
        "out": nc.dram_tensor("out", [S, HIDDEN], f32, kind="ExternalOutput"),
    }
    with tile.TileContext(nc) as tc:
        _emit(nc, tc, tens)
    nc.compile()
    return nc


def get_program():
    if "nc" not in _CACHE:
        _CACHE["nc"] = _build()
    return _CACHE["nc"]


def make_in_maps(hidden_states, attention_mask, wq, wk, wv, wo):
    """Build the per-core input maps (host-side sharding, bf16 cast)."""
    import ml_dtypes
    bf = ml_dtypes.bfloat16
    hidden_states = np.asarray(hidden_states, dtype=np.float32)
    attention_mask = np.asarray(attention_mask, dtype=np.float32)
    wq = np.asarray(wq, dtype=np.float32)
    wk = np.asarray(wk, dtype=np.float32)
    wv = np.asarray(wv, dtype=np.float32)
    wo = np.asarray(wo, dtype=np.float32)

    # Pair-level mask tiles for the diagonal blocks of scores^T, derived from
    # the provided additive mask (0 = attend, big negative = blocked).
    # Tile [t][kk, 512u + qq] = allow(q = 512 + qq, k = 512 + (2t+u)*128 + kk).
    am = attention_mask[0, 0]
    mask_np = np.empty((2, P, 1024), dtype=np.float32)
    for t in range(2):
        for u in range(2):
            off = (2 * t + u) * P
            blk = (am[512:1024, 512 + off:512 + off + P] == 0.0)
            mask_np[t, :, u * 512:(u + 1) * 512] = blk.T.astype(np.float32)
    mask_np = mask_np.astype(bf)

    in_maps = []
    for c in range(NCORES):
        b, g = divmod(c, HG)
        rows = slice(g * DG, (g + 1) * DG)
        in_maps.append({
            "xT": np.ascontiguousarray(hidden_states[b].T).astype(bf),
            "wqT": np.ascontiguousarray(wq[rows, :].T).astype(bf),
            "wkT": np.ascontiguousarray(wk[rows, :].T).astype(bf),
            "wvT": np.ascontiguousarray(wv[rows, :].T).astype(bf),
            "woT": np.ascontiguousarray(wo[:, rows].T).astype(bf),
            "masks": mask_np,
        })
    return in_maps


def combine_outputs(results):
    out = np.empty((B, S, HIDDEN), dtype=np.float32)
    for b in range(B):
        out[b] = results[HG * b]["out"] + results[HG * b + 1]["out"]
    return out


def kernel(hidden_states, attention_mask, wq, wk, wv, wo):
    from concourse.bass_utils import run_bass_kernel_spmd

    nc = get_program()
    in_maps = make_in_maps(hidden_states, attention_mask, wq, wk, wv, wo)
    res = run_bass_kernel_spmd(nc, in_maps, list(range(NCORES)))
    return combine_outputs(res.results)


# revision 23
# speedup vs baseline: 1.1361x; 1.0098x over previous
"""Trainium2 Bass kernel for BaseAttention (B=4, S=2048, H=16 heads x 64).

Sharding: 8 cores = 4 batches x 2 head-groups (8 heads / 512 dims each).
Each core computes q/k/v projections for its head group on its batch,
flash-style causal attention (scores never leave the chip), and a partial
o-projection over its 512 head dims. The host sums the two partial outputs
per batch.

All matmul operands are bf16 (inputs cast on host): the PE streams one
moving column per cycle regardless of dtype, but bf16 halves SBUF/DMA
traffic and weight-load time. Accumulation stays fp32 in PSUM; measured
end-to-end error vs the fp32 reference is ~5e-3 max-rel. The attention
matrix never leaves SBUF: normalized probs land in attnT_sb (bf16) and
the o-projection consumes them directly.

Softmax denominators come from a ones column appended to V; normalization
broadcasts the denominator row with a K=1 matmul, takes the reciprocal on
DVE, and scales attn^T while writing bf16 into attnT_sb.
"""

import numpy as np

B = 4
S = 2048
HIDDEN = 1024
NH = 16
DH = 64
HG = 2                  # head groups (cores per batch)
DG = HIDDEN // HG       # 512 dims per group (8 heads)
NCORES = B * HG
SCALE = DH ** -0.5

P = 128
KC = HIDDEN // P        # 8 contraction chunks for projections
NQ = S // 512           # 4 query chunks of 512
SM = S // P             # 16 seq chunks of 128
MCH = DG // P           # 4 chunks of 128 over the group's 512 dims
NHG = NH // HG          # 8 heads per core
NJ = NHG // 2           # 4 head pairs per core

_CACHE = {}


def _emit(nc, tc, tens):
    import concourse.mybir as mybir
    import concourse.bass as bass
    from collections import deque
    from contextlib import ExitStack

    f32 = mybir.dt.float32
    f32r = mybir.dt.float32r
    bf16 = mybir.dt.bfloat16
    Exp = mybir.ActivationFunctionType.Exp
    mult = mybir.AluOpType.mult
    add = mybir.AluOpType.add
    ds = bass.ds

    xT = tens["xT"].ap().rearrange("(kc p) s -> p kc s", p=P)
    wqT = tens["wqT"].ap().rearrange("(kc p) d -> p kc d", p=P)
    wkT = tens["wkT"].ap().rearrange("(kc p) d -> p kc d", p=P)
    wvT = tens["wvT"].ap().rearrange("(kc p) d -> p kc d", p=P)
    woT = tens["woT"].ap().rearrange("(ic p) j -> p ic j", p=P)
    masks = tens["masks"].ap().rearrange("t p q -> p t q")
    out = tens["out"].ap().rearrange("(sm p) j -> p sm j", p=P)

    with ExitStack() as ctx:
        persist = ctx.enter_context(tc.tile_pool(name="persist", bufs=1))
        ps_mm = ctx.enter_context(tc.tile_pool(name="ps_mm", bufs=3, space="PSUM"))
        ps_at = ctx.enter_context(tc.tile_pool(name="ps_at", bufs=2, space="PSUM"))
        pstage = ctx.enter_context(tc.tile_pool(name="pstage", bufs=2))
        ptp = ctx.enter_context(tc.tile_pool(name="pt", bufs=4))
        recp = ctx.enter_context(tc.tile_pool(name="rec", bufs=2))
        ostp = ctx.enter_context(tc.tile_pool(name="ost", bufs=3))
        ostA = ctx.enter_context(tc.tile_pool(name="ostA", bufs=8))
        qpool = ctx.enter_context(tc.tile_pool(name="qp", bufs=2))

        kT_sb = persist.tile([P, MCH, S], bf16)          # k^T (d on partitions)
        v_sb = persist.tile([P, SM, NHG, DH + 1], bf16)  # v + ones column
        ones_sb = persist.tile([P, DH], f32r)
        wq_sb = persist.tile([P, KC, DG], bf16)
        wk_sb = persist.tile([P, KC, DG], bf16)
        wv_sb = persist.tile([P, KC, DG], bf16)
        wo_sb = persist.tile([P, MCH, HIDDEN], bf16)
        mask_sb = persist.tile([P, 2, 1024], bf16)
        attnT_sb = persist.tile([P, MCH, S], bf16)       # normalized attn^T

        ones_f32 = persist.tile([P, 1], f32)
        nc.vector.memset(ones_f32[:], 1.0)  # f32r memset fails ISA checks
        nc.vector.tensor_copy(out=ones_sb[:], in_=ones_f32[:, 0:1].to_broadcast([P, DH]))
        nc.vector.tensor_copy(
            out=v_sb[:, :, :, DH:DH + 1],
            in_=ones_f32[:, 0:1].to_broadcast([P, SM, NHG, 1]),
        )

        xts = {}
        qsbs = {}

        def proj_closures(n):
            """q/k/v projection work for seq chunk n: 13 closures."""
            cls = []

            def load_xt(n=n):
                xt = pstage.tile([P, KC, 512], bf16, tag="xt")
                # fine-grained: one DMA per kc (128KB) for queue balance and
                # so the first q chain can start as soon as early chunks land
                for kc in range(KC):
                    nc.sync.dma_start(xt[:, kc, :], xT[:, kc, ds(n * 512, 512)])
                xts[n] = xt
                qsbs[n] = qpool.tile([P, MCH, 512], bf16, tag="qsb", name=f"qsb_{n}")
            cls.append(load_xt)

            for w_sb, dst_name in ((wq_sb, "q"), (wk_sb, "k")):
                for m in range(MCH):
                    def qk_group(n=n, w_sb=w_sb, dst_name=dst_name, m=m):
                        xt = xts[n]
                        ps = ps_mm.tile([P, 1024], f32, tag="mm")
                        for kc in range(KC):
                            nc.tensor.matmul(
                                ps[:, :512],
                                w_sb[:, kc, ds(m * P, P)],
                                xt[:, kc, :],
                                start=(kc == 0), stop=(kc == KC - 1),
                            )
                        if dst_name == "k":
                            nc.vector.tensor_copy(
                                out=kT_sb[:, m, ds(n * 512, 512)], in_=ps[:, :512]
                            )
                        else:
                            nc.vector.tensor_copy(
                                out=qsbs[n][:, m, :], in_=ps[:, :512]
                            )
                    cls.append(qk_group)

            for sm in range(4 * n, 4 * n + 4):
                def v_group(n=n, sm=sm):
                    xt = xts[n]
                    ps = ps_mm.tile([P, 1024], f32, tag="mm")
                    for kc in range(KC):
                        nc.tensor.matmul(
                            ps[:, :512],
                            xt[:, kc, ds((sm - 4 * n) * P, P)],
                            wv_sb[:, kc, :],
                            start=(kc == 0), stop=(kc == KC - 1),
                        )
                    nc.vector.tensor_copy(
                        out=v_sb[:, sm, :, 0:DH],
                        in_=ps[:, :512].rearrange("p (h d) -> p h d", h=NHG),
                    )
                cls.append(v_group)
            return cls

        def oproj_closures(n):
            """o-projection for seq chunk n, reading attn^T from SBUF."""
            cls = []
            for sm in range(4 * n, 4 * n + 4):
                for j2 in range(2):
                    def o_group(sm=sm, j2=j2):
                        ps = ps_mm.tile([P, 1024], f32, tag="mm")
                        for ic in range(MCH):
                            nc.tensor.matmul(
                                ps[:, :512],
                                attnT_sb[:, ic, ds(sm * P, P)],
                                wo_sb[:, ic, ds(j2 * 512, 512)],
                                start=(ic == 0), stop=(ic == MCH - 1),
                            )
                        ost = ostp.tile([P, 512], f32, tag="ost")
                        nc.vector.tensor_copy(out=ost[:], in_=ps[:, :512])
                        nc.sync.dma_start(out[:, sm, ds(j2 * 512, 512)], ost[:])
                    cls.append(o_group)
            return cls

        st_half = {}

        def oproj_last_closures():
            """Chunk-3 o-proj split: ic0-1 partials run as filler during the
            last head pairs; only the ic2-3 half serializes after the final
            norm, shrinking the tail."""
            clsA, clsB = [], []
            for sm in range(12, 16):
                for j2 in range(2):
                    def a_group(sm=sm, j2=j2):
                        ps = ps_mm.tile([P, 1024], f32, tag="mm")
                        for ic in (0, 1):
                            nc.tensor.matmul(
                                ps[:, :512],
                                attnT_sb[:, ic, ds(sm * P, P)],
                                wo_sb[:, ic, ds(j2 * 512, 512)],
                                start=(ic == 0), stop=(ic == 1),
                            )
                        st = ostA.tile([P, 512], f32, tag="stA",
                                       name=f"stA_{sm}_{j2}")
                        nc.vector.tensor_copy(out=st[:], in_=ps[:, :512])
                        st_half[(sm, j2)] = st
                    clsA.append(a_group)

                    def b_group(sm=sm, j2=j2):
                        ps = ps_mm.tile([P, 1024], f32, tag="mm")
                        for ic in (2, 3):
                            nc.tensor.matmul(
                                ps[:, :512],
                                attnT_sb[:, ic, ds(sm * P, P)],
                                wo_sb[:, ic, ds(j2 * 512, 512)],
                                start=(ic == 2), stop=(ic == 3),
                            )
                        ost = ostp.tile([P, 512], f32, tag="ost")
                        nc.vector.tensor_tensor(
                            ost[:], ps[:, :512], st_half[(sm, j2)][:], add)
                        nc.sync.dma_start(out[:, sm, ds(j2 * 512, 512)], ost[:])
                    clsB.append(b_group)
            return clsA, clsB

        # startup: xt(0)+wq first so the PE starts within a few us; the
        # remaining weight DMAs stream behind the first matmul groups.
        def wload(dst, src, ncol, nsplit, kdim=KC):
            # split a [*, kdim, ncol] weight load into 128KB pieces,
            # column-major so earlier-needed columns land first
            for c in range(ncol // P):
                csl = ds(c * P, P)
                for h in range(nsplit):
                    ksl = ds(h * (kdim // nsplit), kdim // nsplit)
                    nc.sync.dma_start(dst[:, ksl, csl], src[:, ksl, csl])

        # startup: issue DMAs in consumption order, 128KB pieces so no
        # single queue serializes a dependency (q m1 @~4us, mask @~6us, ...)
        p0 = proj_closures(0)
        p0[0]()                                   # xt(0) + qsb alloc
        wload(wq_sb, wqT, P, 2)                   # wq m0 column only
        p0[1]()                                   # q m0 group
        for c4 in range(1, 4):
            csl = ds(c4 * P, P)
            for h2 in range(2):
                ksl = ds(h2 * (KC // 2), KC // 2)
                nc.sync.dma_start(wq_sb[:, ksl, csl], wqT[:, ksl, csl])
        for q4 in range(4):
            nc.sync.dma_start(mask_sb[:, :, ds(q4 * 256, 256)],
                              masks[:, :, ds(q4 * 256, 256)])
        for c in p0[2:5]:                         # q m1..3 groups
            c()
        wload(wk_sb, wkT, DG, 2)
        for c in p0[5:9]:                         # k groups
            c()
        wload(wv_sb, wvT, DG, 2)
        for c in p0[9:]:                          # v groups
            c()
        wload(wo_sb, woT, HIDDEN, 1, kdim=MCH)

        filler = deque()
        pending = []  # deferred normalization closures

        def flush_pending():
            for c in pending:
                c()
            pending.clear()

        def norm_closure(n, j, acc):
            qsl = ds(n * 512, 512)
            # copy both raw denominator rows to SBUF (f32r) as bcast-mm rhs
            rec = recp.tile([DH + 1, 1024], f32r, tag="rec")
            with nc.allow_low_precision(reason="denom row stage"):
                for e in range(2):
                    nc.vector.tensor_copy(
                        out=rec[DH:DH + 1, ds(e * 512, 512)],
                        in_=acc[e][DH:DH + 1, :],
                    )

            def finish():
                bc = ps_mm.tile([P, 1024], f32, tag="mm", name=f"bc_{n}_{j}")
                for e in range(2):
                    nc.tensor.matmul(
                        bc[0:DH, ds(e * 512, 512)], ones_sb[DH:DH + 1, :],
                        rec[DH:DH + 1, ds(e * 512, 512)], start=True, stop=True,
                    )
                rcp = recp.tile([DH, 1024], f32, tag="rcp")
                nc.vector.reciprocal_approx_fast(rcp[:], bc[0:DH, :])
                with nc.allow_low_precision(reason="attn bf16 store"):
                    for e in range(2):
                        nc.vector.tensor_tensor(
                            attnT_sb[ds(64 * e, DH), j, qsl], acc[e][0:DH, :],
                            rcp[:, ds(e * 512, 512)], mult,
                        )
            return finish

        for n in range(NQ):
            if n + 1 < NQ:
                pc = proj_closures(n + 1)
                pc[0]()                       # start xt(n+1) DMA immediately
                filler.extend(pc[1:])
            if 1 <= n <= 3:
                filler.extend(oproj_closures(n - 1))
            if n == 3:
                lastA, lastB = oproj_last_closures()
                filler.extend(lastA)
            npairs = 2 * (n + 1)
            total_pairs = NJ * npairs
            pace_num = len(filler)
            pace_acc = 0
            for j in range(NJ):
                acc = [
                    ps_at.tile([DH + 1, 512], f32, tag="acc",
                               name=f"acc_{n}_{j}_{e}")
                    for e in range(2)
                ]
                pvq = []  # deferred PV matmuls (consumed 2 pairs later)

                def emit_pv():
                    tp, e, u, pt, qoff = pvq.pop(0)
                    h = 2 * j + e
                    m = 2 * tp + u
                    nc.tensor.matmul(
                        acc[e][:, qoff:512],
                        v_sb[:, m, h, :],
                        pt[:, ds(u * 512 + qoff, 512 - qoff)],
                        start=(tp == 0 and u == 0),
                        stop=(tp == npairs - 1 and u == 1),
                        skip_group_check=(qoff > 0),
                    )

                for t in range(npairs):
                    if pending and t == 1:
                        flush_pending()
                    # pump interleaved proj/o-proj work in bursts of >=2
                    # groups: a dense >3.4us PE stretch lets the HAM clock
                    # gate open (scattered 1-group pumps never do)
                    pace_acc += pace_num
                    while pace_acc >= total_pairs and filler:
                        filler.popleft()()
                        pace_acc -= total_pairs
                    diag = t - 2 * n            # >=0: diagonal pair
                    # causal trim: within the diagonal 512-block, q columns
                    # strictly left of a key block are fully masked — skip
                    # their scores/exp/PV entirely (start flags still cover
                    # the full width via the t==0,u==0 PV).
                    qoffs = ((0, 128) if diag == 0 else
                             (256, 384) if diag == 1 else (0, 0))
                    new_pvq = []
                    for e in range(2):          # head pair member
                        bp = e * DH             # base partition 0/64
                        ps = ps_mm.tile([P, 1024], f32, tag="mm")
                        for u in range(2):      # m-pair member
                            m = 2 * t + u
                            qo = qoffs[u]
                            nc.tensor.matmul(
                                ps[:, ds(u * 512 + qo, 512 - qo)],
                                kT_sb[bp:bp + DH, j, ds(m * P, P)],
                                qsbs[n][bp:bp + DH, j, ds(qo, 512 - qo)],
                                start=True, stop=True,
                            )
                            if pvq and pvq[0][0] <= t - 2:
                                emit_pv()
                        pt = ptp.tile([P, 1024], bf16, tag="pt")
                        if diag == 1:
                            nc.scalar.activation(pt[:, 256:512], ps[:, 256:512],
                                                 Exp, scale=SCALE)
                            nc.scalar.activation(pt[:, 896:1024], ps[:, 896:1024],
                                                 Exp, scale=SCALE)
                        else:
                            nc.scalar.activation(pt[:], ps[:], Exp, scale=SCALE)
                        if diag == 0:
                            nc.vector.tensor_tensor(
                                pt[:, 0:128], pt[:, 0:128],
                                mask_sb[:, 0, 0:128], mult)
                            nc.vector.tensor_tensor(
                                pt[:, 640:768], pt[:, 640:768],
                                mask_sb[:, 0, 640:768], mult)
                        elif diag == 1:
                            nc.vector.tensor_tensor(
                                pt[:, 256:384], pt[:, 256:384],
                                mask_sb[:, 1, 256:384], mult)
                            nc.vector.tensor_tensor(
                                pt[:, 896:1024], pt[:, 896:1024],
                                mask_sb[:, 1, 896:1024], mult)
                        if pvq and pvq[0][0] <= t - 2:
                            emit_pv()
                        new_pvq.extend((t, e, u, pt, qoffs[u]) for u in range(2))
                    while pvq and pvq[0][0] <= t - 2:
                        emit_pv()
                    pvq.extend(new_pvq)
                while pvq:
                    emit_pv()
                pending.append(norm_closure(n, j, acc))
            while filler:
                filler.popleft()()
        flush_pending()
        for c in lastB:
            c()


def _build():
    import concourse.mybir as mybir
    import concourse.tile as tile
    from concourse import bacc

    f32 = mybir.dt.float32
    bf16 = mybir.dt.bfloat16
    nc = bacc.Bacc("TRN2", target_bir_lowering=False, debug=False,
                   num_devices=NCORES)
    tens = {
        "xT": nc.dram_tensor("xT", [HIDDEN, S], bf16, kind="ExternalInput"),
        "wqT": nc.dram_tensor("wqT", [HIDDEN, DG], bf16, kind="ExternalInput"),
        "wkT": nc.dram_tensor("wkT", [HIDDEN, DG], bf16, kind="ExternalInput"),
        "wvT": nc.dram_tensor("wvT", [HIDDEN, DG], bf16, kind="ExternalInput"),
        "woT": nc.dram_tensor("woT", [DG, HIDDEN], bf16, kind="ExternalInput"),
        "masks": nc.dram_tensor("masks", [2, P, 1024], bf16, kind="ExternalInput"),
        "out": nc.dram_tensor("out", [S, HIDDEN], f32, kind="ExternalOutput"),
    }
    with tile.TileContext(nc) as tc:
        _emit(nc, tc, tens)
    nc.compile()
    return nc


def get_program():
    if "nc" not in _CACHE:
        _CACHE["nc"] = _build()
    return _CACHE["nc"]


def make_in_maps(hidden_states, attention_mask, wq, wk, wv, wo):
    """Build the per-core input maps (host-side sharding, bf16 cast)."""
    import ml_dtypes
    bf = ml_dtypes.bfloat16
    hidden_states = np.asarray(hidden_states, dtype=np.float32)
    attention_mask = np.asarray(attention_mask, dtype=np.float32)
    wq = np.asarray(wq, dtype=np.float32)
    wk = np.asarray(wk, dtype=np.float32)
    wv = np.asarray(wv, dtype=np.float32)
    wo = np.asarray(wo, dtype=np.float32)

    # Pair-level mask tiles for the diagonal blocks of scores^T, derived from
    # the provided additive mask (0 = attend, big negative = blocked).
    # Tile [t][kk, 512u + qq] = allow(q = 512 + qq, k = 512 + (2t+u)*128 + kk).
    am = attention_mask[0, 0]
    mask_np = np.empty((2, P, 1024), dtype=np.float32)
    for t in range(2):
        for u in range(2):
            off = (2 * t + u) * P
            blk = (am[512:1024, 512 + off:512 + off + P] == 0.0)
            mask_np[t, :, u * 512:(u + 1) * 512] = blk.T.astype(np.float32)
    mask_np = mask_np.astype(bf)

    in_maps = []
    for c in range(NCORES):
        b, g = divmod(c, HG)
        rows = slice(g * DG, (g + 1) * DG)
        in_maps.append({
            "xT": np.ascontiguousarray(hidden_states[b].T).astype(bf),
            "wqT": np.ascontiguousarray(wq[rows, :].T).astype(bf),
            "wkT": np.ascontiguousarray(wk[rows, :].T).astype(bf),
            "wvT": np.ascontiguousarray(wv[rows, :].T).astype(bf),
            "woT": np.ascontiguousarray(wo[:, rows].T).astype(bf),
            "masks": mask_np,
        })
    return in_maps


def combine_outputs(results):
    out = np.empty((B, S, HIDDEN), dtype=np.float32)
    for b in range(B):
        out[b] = results[HG * b]["out"] + results[HG * b + 1]["out"]
    return out


def kernel(hidden_states, attention_mask, wq, wk, wv, wo):
    from concourse.bass_utils import run_bass_kernel_spmd

    nc = get_program()
    in_maps = make_in_maps(hidden_states, attention_mask, wq, wk, wv, wo)
    res = run_bass_kernel_spmd(nc, in_maps, list(range(NCORES)))
    return combine_outputs(res.results)
